# revision 8
# baseline (speedup 1.0000x reference)
"""Trainium2 Bass kernel for MinibatchDiscrimination2d.

Full computation:
  x (32,128,64,64) --conv s4--> x_r (32,3,16,16)
  M = x_r @ T  -> (32, 8192, 16)
  dist[b1,b2,d] = sum_f |M[b1,d,f]-M[b2,d,f]|
  out[b,d] = sum_b2 exp(-dist) - 1 -> (32,32,16,16)
  out_a = deconv s4 (32,32,64,64); return concat([x, out_a], ch)

Device pipeline (per core, per dgroup g of 128 d-values):
  1. M-step: fp8 DoubleRow matmuls stream the T shard through the PE at
     2 fp8/PE/cycle: psb = xr8^T @ T8 (f32 PSUM) -> Mb bf16 [32, 2048].
     Host pre-transforms T into (T+, T-) f-pair combos: columns are
     (jj 8, sigma 2, s 128) per g, so Mb holds Mpm = D-pair combos.
  2. PE transposes Mb 128-col chunks -> M_T2[sigma] [128 s, (jj 8, b 48)]
     bf16 SBUF (b ring-extended to 48 for cyclic pair addressing).
  3. DVE pairwise (all bf16 SBUF, 2x mode): for the 512 ordered cyclic
     pairs (b1, b1+o), o=1..16:
        Dp = M_T2[+][b1] - M_T2[+][b1+o]   (sliding-window AP)
        Dm = M_T2[-][b1] - M_T2[-][b1+o]
        u  = max(|Dp|, |Dm|) = |D_2j| + |D_2j+1|   (exact identity)
        dist = sum_jj u  (bf16 add tree, final add in f32)
     This replaces both the signed-incidence PE matmuls and the slow
     1-elem/cycle f32-PSUM tensor_reduce of the previous version.
  4. Act: E = exp(-dist/128) bf16; PE transposes E; 4 matmuls with the
     pair->sample incidence matrix accumulate acc[s, b].
  5. Deconv per r-half as before; y bf16, contiguous layout.

The tiny conv (0.008% of FLOPs) runs on the host during input prep,
which removes the cross-core AllGather and its ~50us rendezvous
barrier; the x passthrough half of the output is host-side concat.

Scaling: xr8 = 2*x_r (fp8), T8 = 64*Tpm (fp8) => Mb = 128*Mpm,
dist scaled by 128, exp applies scale=-1/128.
"""

import numpy as np
import ml_dtypes

N_CORES = 8
B, IN_FLT, N = 32, 128, 64
K = 4
T_SP = 16
OC = 32
F = 16
D_IN = 768
DSH = 1024                 # d per core
NG = DSH // 128            # 8 dgroups
KCH = D_IN // 128          # 6 contraction chunks (3 DoubleRow pairs)
NO = 16                    # cyclic pair offsets o = 1..16
NPAIR = NO * B             # 512 ordered cyclic pairs

_CACHE = {}


def _build_nc():
    import concourse.bacc as bacc
    import concourse.mybir as mybir
    import concourse.tile as tile
    from concourse.bass import AP

    f32 = mybir.dt.float32
    bf16 = mybir.dt.bfloat16
    f8 = mybir.dt.float8e4
    AFT = mybir.ActivationFunctionType
    ALU = mybir.AluOpType
    DR = mybir.MatmulPerfMode.DoubleRow
    X = mybir.AxisListType.X

    nc = bacc.Bacc("TRN2", target_bir_lowering=False, debug=False,
                   num_devices=N_CORES)

    tsh8 = nc.dram_tensor("tsh8", [NG * 3 * 128, 2 * 2048], f8,
                          kind="ExternalInput")
    xr8 = nc.dram_tensor("xr8", [128, KCH * B], f8, kind="ExternalInput")
    inc2 = nc.dram_tensor("inc2", [128, 4 * B], bf16, kind="ExternalInput")
    wd = nc.dram_tensor("wd", [OC, 512], bf16, kind="ExternalInput")
    eye32 = nc.dram_tensor("eye32", [B, B], bf16, kind="ExternalInput")
    eye128 = nc.dram_tensor("eye128", [128, 128], bf16, kind="ExternalInput")
    y = nc.dram_tensor("y", [2 * 128, 2048], bf16, kind="ExternalOutput")

    with tile.TileContext(nc) as tc:
        with tc.tile_pool(name="const", bufs=1) as constp, \
             tc.tile_pool(name="Tp", bufs=3 * NG) as Tp, \
             tc.tile_pool(name="Mp", bufs=2) as Mp, \
             tc.tile_pool(name="mt", bufs=4) as mtp, \
             tc.tile_pool(name="dwork", bufs=1) as dwp, \
             tc.tile_pool(name="ew", bufs=2) as ewp, \
             tc.tile_pool(name="persist", bufs=1) as pp, \
             tc.tile_pool(name="ps_m", bufs=2, space="PSUM") as ps_m, \
             tc.tile_pool(name="ps_t", bufs=2, space="PSUM") as ps_t, \
             tc.tile_pool(name="ps_e", bufs=1, space="PSUM") as ps_e, \
             tc.tile_pool(name="ps_acc", bufs=1, space="PSUM") as ps_acc, \
             tc.tile_pool(name="ps_dec", bufs=1, space="PSUM") as ps_dec:

            xr_sb = constp.tile([128, KCH * B], f8)
            nc.scalar.dma_start(xr_sb[:], xr8[:])
            inc_sb = constp.tile([128, 4, B], bf16)
            nc.scalar.dma_start(inc_sb[:].rearrange("p q b -> p (q b)"), inc2[:])
            wd_sb = constp.tile([OC, 512], bf16)
            nc.scalar.dma_start(wd_sb[:], wd[:])
            e32_sb = constp.tile([B, B], bf16)
            nc.scalar.dma_start(e32_sb[:], eye32[:])
            e128_sb = constp.tile([128, 128], bf16)
            nc.scalar.dma_start(e128_sb[:], eye128[:])

            # prefetch the whole T shard (24 x 512KB), 2 queues
            Ts = []
            for g in range(NG):
                row = []
                for kp in range(3):
                    t = Tp.tile([128, 2 * 2048], f8, tag="T")
                    eng = nc.sync if (g * 3 + kp) % 2 == 0 else nc.gpsimd
                    r0 = (g * 3 + kp) * 128
                    eng.dma_start(t[:], tsh8[r0:r0 + 128, :])
                    row.append(t)
                Ts.append(row)

            acc = pp.tile([128, NG * B], f32)        # col = g*32 + b
            acc2 = pp.tile([OC, 32 * B], bf16)       # (32 ch, col = rj*32 + b)
            wd_v = wd_sb[:].rearrange("c (m v) -> c v m", v=4)
            xr_v = xr_sb[:].rearrange("p (k b) -> p k b", k=KCH)

            def _deconv_r(r):
                acc2_3 = acc2[:].rearrange("c (g x b) -> c g x b", g=NG, x=4)
                for q in range(4):
                    nc.gpsimd.dma_start(
                        acc2_3[:, 4 * r:4 * r + 4, q, :],
                        acc[q * 32:(q + 1) * 32, 4 * r * B:(4 * r + 4) * B]
                        .rearrange("c (g b) -> c g b", g=4))
                yst = ewp.tile([128, 4, 512], bf16, tag="yst")  # (v, j, b)
                for v in range(4):
                    psdt = ps_dec.tile([128, 512], f32, tag="psD2")
                    nc.tensor.matmul(
                        psdt[:], wd_v[:, v], acc2[:, r * 512:(r + 1) * 512],
                        start=True, stop=True)
                    nc.scalar.copy(yst[:, v, :], psdt[:])
                nc.sync.dma_start(
                    y[r * 128:(r + 1) * 128, :],
                    yst[:].rearrange("p v b -> p (v b)"))

            def _sigma_stage(g, dist):
                # dist [128 s, 512 pairs] f32 -> acc[:, g*32:(g+1)*32]
                E = ewp.tile([128, NPAIR], bf16, tag="E")
                nc.scalar.activation(E[:], dist[:], AFT.Exp, scale=-1.0 / 128.0)
                psE = ps_e.tile([128, 4, 128], bf16, tag="psE")
                for q in range(4):
                    nc.tensor.transpose(
                        psE[:, q, :], E[:, q * 128:(q + 1) * 128], e128_sb[:])
                E_T = ewp.tile([128, 4, 128], bf16, tag="ET")
                nc.scalar.copy(E_T[:].rearrange("p q s -> p (q s)"),
                               psE[:].rearrange("p q s -> p (q s)"))
                accg = ps_acc.tile([128, B], f32, tag="accg")
                for q in range(4):
                    nc.tensor.matmul(
                        accg[:], E_T[:, q, :], inc_sb[:, q, :],
                        start=(q == 0), stop=(q == 3))
                nc.scalar.copy(acc[:, g * B:(g + 1) * B], accg[:])

            dists = [None] * NG
            for g in range(NG):
                # ---- M-step: fp8 DR matmuls; Mb cols = (jj 8, sg 2, s 128)
                Mb = Mp.tile([B, 2048], bf16, tag="M")
                for ncn in range(4):
                    psb = ps_m.tile([B, 512], f32, tag="psM")
                    for kp in range(3):
                        nc.tensor.matmul(
                            psb[:],
                            xr_v[:, 2 * kp:2 * kp + 2, :],
                            Ts[g][kp][:]
                            .rearrange("p (i n) -> p i n", i=2)
                            [:, :, ncn * 512:(ncn + 1) * 512],
                            start=(kp == 0), stop=(kp == 2),
                            perf_mode=DR)
                    nc.scalar.copy(Mb[:, ncn * 512:(ncn + 1) * 512], psb[:])

                # ---- transposes: M_T2[sg] [128 s, (jj 8, b 48)] bf16
                MT = []
                for sg in range(2):
                    psT = ps_t.tile([128, 8, B], bf16, tag="psT")
                    for jj in range(8):
                        jc = jj * 2 + sg
                        nc.tensor.transpose(
                            psT[:, jj, :],
                            Mb[:, jc * 128:(jc + 1) * 128], e32_sb[:])
                    mt2 = mtp.tile([128, 8, 48], bf16, tag="MT")
                    nc.scalar.copy(mt2[:, :, 0:B], psT[:])
                    nc.scalar.copy(mt2[:, :, B:B + 16], psT[:, :, 0:16])
                    MT.append(mt2)

                # ---- DVE pairwise: 512 cyclic ordered pairs
                # u = max(|Dp|,|Dm|) = |D_2j|+|D_2j+1|; abs_max is not
                # supported by codegen, so: Act computes |Dp|, then
                # t = max(-Dm, |Dp|) (STT mult/max), u = max(t, Dm).
                Dp = dwp.tile([128, 8, NO, B], bf16, tag="Dp")
                Dm = dwp.tile([128, 8, NO, B], bf16, tag="Dm")
                aDp = dwp.tile([128, 8, NO, B], bf16, tag="aDp")
                tmx = dwp.tile([128, 8, NO, B], bf16, tag="tmx")
                u = dwp.tile([128, 8, NO, B], bf16, tag="u")
                with nc.allow_low_precision("bf16 pairwise tree, f32 final"):
                    for sg, Dt in ((0, Dp), (1, Dm)):
                        a = MT[sg][:]
                        in0 = (a[:, :, 0:B].unsqueeze(2)
                               .broadcast_to([128, 8, NO, B]))
                        in1 = AP(a.tensor, a.offset + 1,
                                 [list(a.ap[0]), [48, 8], [1, NO], [1, B]])
                        nc.vector.tensor_tensor(Dt[:], in0, in1, ALU.subtract)
                    nc.scalar.activation(aDp[:], Dp[:], AFT.Abs)
                    nc.vector.scalar_tensor_tensor(
                        tmx[:], Dm[:], -1.0, aDp[:], op0=ALU.mult, op1=ALU.max)
                    nc.vector.tensor_tensor(u[:], tmx[:], Dm[:], ALU.max)
                    uv = u[:].rearrange("p j o b -> p j (o b)")
                    v2t = dwp.tile([128, 2, 2, NPAIR], bf16, tag="v2")
                    nc.vector.tensor_tensor(
                        v2t[:].rearrange("p a c n -> p (a c) n"),
                        uv[:, 0:4, :], uv[:, 4:8, :], ALU.add)
                    w2 = dwp.tile([128, 2, NPAIR], bf16, tag="w2")
                    nc.vector.tensor_tensor(w2[:], v2t[:, 0], v2t[:, 1], ALU.add)
                    dist = ewp.tile([128, NPAIR], f32, tag="dist")
                    nc.vector.tensor_tensor(dist[:], w2[:, 0], w2[:, 1], ALU.add)

                dists[g] = dist

                # ---- software-pipelined sigma/deconv stages (lag 1)
                if g >= 1:
                    _sigma_stage(g - 1, dists[g - 1][:])
                    dists[g - 1] = None
                if g == NG // 2:
                    _deconv_r(0)
            _sigma_stage(NG - 1, dists[NG - 1][:])
            _deconv_r(1)

    nc.finalize()
    return nc


def _np_f8():
    import concourse.mybir as mybir
    return mybir.dt.np(mybir.dt.float8e4)


def _host_prep(x, w_conv, T, w_deconv):
    """Host-side prep: conv, T+- transform, scaling, fp8 casts, shards."""
    bf = ml_dtypes.bfloat16
    f8 = _np_f8()

    # conv (stride 4, VALID) on host: x_r (32, 768) f32
    xw = np.ascontiguousarray(x, dtype=np.float32).reshape(B, IN_FLT, 16, K, 16, K)
    x_r = np.einsum('bcirjs,ocrs->boij', xw, w_conv.astype(np.float32),
                    optimize=True).reshape(B, D_IN)
    xr3 = (2.0 * x_r.T).reshape(KCH, 128, B).transpose(1, 0, 2)
    xr8_host = np.ascontiguousarray(xr3).reshape(128, KCH * B).astype(f8)

    # deconv weights: lhsT[ic, (u*32+oc)*4+v] = w_deconv[oc, ic, u, v]
    wd_host = np.ascontiguousarray(
        np.transpose(w_deconv, (1, 2, 0, 3)).reshape(OC, 512)).astype(bf)

    # cyclic-pair incidence: pair p = (o-1)*32 + b1, o = 1..16
    inc_host = np.zeros((128, 4 * B), np.float32)
    for o in range(1, NO + 1):
        for b1 in range(B):
            p = (o - 1) * B + b1
            q, pw = p // 128, p % 128
            inc_host[pw, q * B + b1] = 1.0
            if o < NO:
                inc_host[pw, q * B + (b1 + o) % B] = 1.0
    inc_host = inc_host.astype(bf)
    eye32_host = np.eye(B, dtype=np.float32).astype(bf)
    eye128_host = np.eye(128, dtype=np.float32).astype(bf)

    # T: (768, 8192, 16) -> (768, 32oc, 16i, 16j, 16f)
    Tr = np.ascontiguousarray(T).reshape(D_IN, OC, T_SP, T_SP, F)
    in_maps = []
    for c in range(N_CORES):
        tslice = Tr[:, :, 2 * c:2 * c + 2, :, :]           # (768, oc, r, j, f)
        # -> (768, rj 32, oc 32, f 16) -> (768, g 8, x 4, oc 32, jj 8, 2)
        A = np.transpose(tslice, (0, 2, 3, 1, 4)).reshape(
            D_IN, NG, 4, OC, 8, 2)
        Apm = np.stack([A[..., 0] + A[..., 1], A[..., 0] - A[..., 1]], axis=4)
        # (768, g, x, oc, sg 2, jj 8) -> order cols (jj, sg, x, oc)
        arr = Apm.transpose(1, 0, 5, 4, 2, 3)   # (g, k, jj, sg, x, oc)
        ts6 = (64.0 * arr).reshape(NG, 3, 2, 128, 2048)  # (g, kp, i, p, col)
        tsh8_host = np.ascontiguousarray(
            ts6.transpose(0, 1, 3, 2, 4)).astype(f8).reshape(NG * 3 * 128, 4096)
        in_maps.append({
            "tsh8": tsh8_host,
            "xr8": xr8_host,
            "inc2": inc_host,
            "wd": wd_host,
            "eye32": eye32_host,
            "eye128": eye128_host,
        })
    return in_maps


def _get_nc():
    if "nc" not in _CACHE:
        _CACHE["nc"] = _build_nc()
    return _CACHE["nc"]


def run(inputs, trace=False, trace_kwargs=None):
    """Run on hardware; returns (full_output, BassKernelResults)."""
    from concourse.bass_utils import run_bass_kernel_spmd
    nc = _get_nc()
    in_maps = _host_prep(inputs["x"], inputs["w_conv"], inputs["T"],
                         inputs["w_deconv"])
    res = run_bass_kernel_spmd(nc, in_maps, list(range(N_CORES)), trace=trace,
                               **(trace_kwargs or {}))
    x = np.asarray(inputs["x"], dtype=np.float32)
    full = np.empty((B, IN_FLT + OC, N, N), np.float32)
    full[:, :IN_FLT] = x
    for c in range(N_CORES):
        yc = np.asarray(res.results[c]["y"], dtype=np.float32)  # (256, 2048)
        for r in range(2):
            arr = yc[r * 128:(r + 1) * 128].reshape(4, 32, 4, 16, 32)
            # [u, o, v, j, b] -> [b, o, u, (j v)]
            full[:, IN_FLT:, 8 * c + 4 * r:8 * c + 4 * r + 4, :] = (
                arr.transpose(4, 1, 0, 3, 2).reshape(B, OC, 4, N))
    return full, res


def kernel(**inputs) -> np.ndarray:
    out, _ = run(inputs, trace=False)
    return out


# revision 17
# speedup vs baseline: 1.0430x; 1.0430x over previous
"""Trainium2 Bass kernel for MinibatchDiscrimination2d.

Full computation:
  x (32,128,64,64) --conv s4--> x_r (32,3,16,16)
  M = x_r @ T  -> (32, 8192, 16)
  dist[b1,b2,d] = sum_f |M[b1,d,f]-M[b2,d,f]|
  out[b,d] = sum_b2 exp(-dist) - 1 -> (32,32,16,16)
  out_a = deconv s4 (32,32,64,64); return concat([x, out_a], ch)

Device pipeline (per core, per dgroup g of 128 d-values):
  1. M-step: fp8 DoubleRow matmuls stream the T shard through the PE at
     2 fp8/PE/cycle: psb = xr8^T @ T8 (f32 PSUM) -> Mb bf16 [32, 2048].
     Host pre-transforms T into (T+, T-) f-pair combos: columns are
     (jj 8, sigma 2, s 128) per g, so Mb holds the D-pair combos.
  2. PE transposes Mb 128-col chunks into psT[sigma] [128 s, (jj 8,
     b 48)] bf16 *in PSUM*; a small Act copy ring-extends b by 16 for
     cyclic pair addressing. DVE reads bf16 PSUM at 2x.
  3. DVE pairwise (2x mode): for the 512 ordered cyclic pairs
     (b1, b1+o), o=1..16:
        Dp = psT[+][b1] - psT[+][b1+o]   (sliding-window AP)
        Dm = psT[-][b1] - psT[-][b1+o]
        u  = max(|Dp|, |Dm|) = |D_2j| + |D_2j+1|   (exact identity)
        dist = sum_jj u  (bf16 add tree, final add in f32)
     |Dp| comes from the Act engine (Abs activation); |Dm| from a DVE
     bitwise_and with 0x7fff via uint16 bitcast (abs_max is not
     supported by the codegen; these two are, and stay at DVE 2x).
  4. sigma-stage (software-pipelined, lag 1): Act E = exp(-dist/128);
     PE transposes E; 4 matmuls with the pair->sample incidence matrix
     accumulate acc[s, b].
  5. Incremental deconv: each dgroup's 128 acc columns are immediately
     shuffled (gpsimd DMA), deconv'd (4 matmuls), and DMA'd out (y
     bf16), so the tail after the last dgroup is tiny.

The tiny conv (0.008% of FLOPs) runs on the host during input prep,
which removes the cross-core AllGather and its ~50us rendezvous
barrier; the x passthrough half of the output is host-side concat.

Scaling: xr8 = 2*x_r (fp8), T8 = 64*Tpm (fp8) => Mb = 128*Mpm,
dist scaled by 128, exp applies scale=-1/128.
"""

import numpy as np
import ml_dtypes

N_CORES = 8
B, IN_FLT, N = 32, 128, 64
K = 4
T_SP = 16
OC = 32
F = 16
D_IN = 768
DSH = 1024                 # d per core
NG = DSH // 128            # 8 dgroups
KCH = D_IN // 128          # 6 contraction chunks (3 DoubleRow pairs)
NO = 16                    # cyclic pair offsets o = 1..16
NPAIR = NO * B             # 512 ordered cyclic pairs

_CACHE = {}


def _build_nc():
    import concourse.bacc as bacc
    import concourse.mybir as mybir
    import concourse.tile as tile
    from concourse.bass import AP

    f32 = mybir.dt.float32
    bf16 = mybir.dt.bfloat16
    f8 = mybir.dt.float8e4
    u16 = mybir.dt.uint16
    AFT = mybir.ActivationFunctionType
    ALU = mybir.AluOpType
    DR = mybir.MatmulPerfMode.DoubleRow

    nc = bacc.Bacc("TRN2", target_bir_lowering=False, debug=False,
                   num_devices=N_CORES)

    tsh8 = nc.dram_tensor("tsh8", [NG * 3 * 128, 2 * 2048], f8,
                          kind="ExternalInput")
    xr8 = nc.dram_tensor("xr8", [128, KCH * B], f8, kind="ExternalInput")
    inc2 = nc.dram_tensor("inc2", [128, 4 * B], bf16, kind="ExternalInput")
    wd = nc.dram_tensor("wd", [OC, 512], bf16, kind="ExternalInput")
    eye32 = nc.dram_tensor("eye32", [B, B], bf16, kind="ExternalInput")
    eye128 = nc.dram_tensor("eye128", [128, 128], bf16, kind="ExternalInput")
    eye16 = nc.dram_tensor("eye16", [16, 16], bf16, kind="ExternalInput")
    mskd = nc.dram_tensor("mskd", [128, B], u16, kind="ExternalInput")
    y = nc.dram_tensor("y", [2 * 128, 2048], bf16, kind="ExternalOutput")

    with tile.TileContext(nc) as tc:
        with tc.tile_pool(name="const", bufs=1) as constp, \
             tc.tile_pool(name="Tp", bufs=3 * NG) as Tp, \
             tc.tile_pool(name="Mp", bufs=2) as Mp, \
             tc.tile_pool(name="dwork", bufs=1) as dwp, \
             tc.tile_pool(name="ew", bufs=2) as ewp, \
             tc.tile_pool(name="persist", bufs=1) as pp, \
             tc.tile_pool(name="ps_m", bufs=1, space="PSUM") as ps_m, \
             tc.tile_pool(name="ps_t", bufs=2, space="PSUM") as ps_t, \
             tc.tile_pool(name="ps_e", bufs=1, space="PSUM") as ps_e, \
             tc.tile_pool(name="ps_acc", bufs=1, space="PSUM") as ps_acc, \
             tc.tile_pool(name="ps_dec", bufs=1, space="PSUM") as ps_dec:

            xr_sb = constp.tile([128, KCH * B], f8)
            nc.scalar.dma_start(xr_sb[:], xr8[:])
            inc_sb = constp.tile([128, 4, B], bf16)
            nc.scalar.dma_start(inc_sb[:].rearrange("p q b -> p (q b)"), inc2[:])
            wd_sb = constp.tile([OC, 512], bf16)
            nc.scalar.dma_start(wd_sb[:], wd[:])
            e32_sb = constp.tile([B, B], bf16)
            nc.scalar.dma_start(e32_sb[:], eye32[:])
            e128_sb = constp.tile([128, 128], bf16)
            nc.scalar.dma_start(e128_sb[:], eye128[:])
            e16_sb = constp.tile([16, 16], bf16)
            nc.scalar.dma_start(e16_sb[:], eye16[:])
            msk_sb = constp.tile([128, B], u16)
            nc.scalar.dma_start(msk_sb[:], mskd[:])

            # prefetch the whole T shard (24 x 512KB); g0 over 3 queues
            qs = [nc.sync, nc.gpsimd, nc.scalar]
            Ts = []
            for g in range(NG):
                row = []
                for kp in range(3):
                    t = Tp.tile([128, 2 * 2048], f8, tag="T")
                    eng = qs[kp % 3] if g == 0 else qs[(g * 3 + kp) % 2]
                    r0 = (g * 3 + kp) * 128
                    eng.dma_start(t[:], tsh8[r0:r0 + 128, :])
                    row.append(t)
                Ts.append(row)

            acc = pp.tile([128, NG * B], f32)        # col = g*32 + b
            acc2 = pp.tile([OC, 32 * B], bf16)       # (32 ch, col = rj*32 + b)
            wd_v = wd_sb[:].rearrange("c (m v) -> c v m", v=4)
            xr_v = xr_sb[:].rearrange("p (k b) -> p k b", k=KCH)
            acc2_4 = acc2[:].rearrange("c (g x b) -> c g x b", g=NG, x=4)
            y_v = y[:].rearrange("p (v j b) -> p v j b", v=4, j=T_SP)

            def _sigma_stage(g, dist):
                # dist [128 s, 512 pairs] f32 -> acc[:, g*32:(g+1)*32]
                E = ewp.tile([128, NPAIR], bf16, tag="E")
                nc.scalar.activation(E[:], dist[:], AFT.Exp, scale=-1.0 / 128.0)
                psE = ps_e.tile([128, 4, 128], bf16, tag="psE")
                for q in range(4):
                    nc.tensor.transpose(
                        psE[:, q, :], E[:, q * 128:(q + 1) * 128], e128_sb[:])
                E_T = ewp.tile([128, 4, 128], bf16, tag="ET")
                nc.scalar.copy(E_T[:].rearrange("p q s -> p (q s)"),
                               psE[:].rearrange("p q s -> p (q s)"))
                accg = ps_acc.tile([128, B], f32, tag="accg")
                for q in range(4):
                    nc.tensor.matmul(
                        accg[:], E_T[:, q, :], inc_sb[:, q, :],
                        start=(q == 0), stop=(q == 3))
                nc.scalar.copy(acc[:, g * B:(g + 1) * B], accg[:])

            def _deconv_g(g):
                # acc cols of dgroup g -> 4 output rows' (j-quarter, v) slab
                r, j0 = g // 4, g % 4
                for q in range(4):
                    nc.gpsimd.dma_start(
                        acc2_4[:, g, q, :],
                        acc[q * 32:(q + 1) * 32, g * B:(g + 1) * B])
                psd4 = ps_dec.tile([128, 4, 128], f32, tag="psd4")
                for v in range(4):
                    nc.tensor.matmul(
                        psd4[:, v, :], wd_v[:, v],
                        acc2[:, g * 128:(g + 1) * 128], start=True, stop=True)
                ystg = ewp.tile([128, 4, 128], bf16, tag="yst")
                nc.scalar.copy(ystg[:].rearrange("p v n -> p (v n)"),
                               psd4[:].rearrange("p v n -> p (v n)"))
                nc.sync.dma_start(
                    y_v[r * 128:(r + 1) * 128, :, 4 * j0:4 * j0 + 4, :],
                    ystg[:].rearrange("p v (j b) -> p v j b", j=4))

            dists = [None] * NG
            for g in range(NG):
                # ---- M-step: fp8 DR matmuls; Mb cols = (jj 8, sg 2, s 128)
                Mb = Mp.tile([B, 2048], bf16, tag="M")
                for h in range(2):
                    psb = ps_m.tile([B, 2, 512], f32, tag="psM")
                    for n2 in range(2):
                        ncn = h * 2 + n2
                        for kp in range(3):
                            nc.tensor.matmul(
                                psb[:, n2, :],
                                xr_v[:, 2 * kp:2 * kp + 2, :],
                                Ts[g][kp][:]
                                .rearrange("p (i n) -> p i n", i=2)
                                [:, :, ncn * 512:(ncn + 1) * 512],
                                start=(kp == 0), stop=(kp == 2),
                                perf_mode=DR)
                    nc.scalar.copy(
                        Mb[:, h * 1024:(h + 1) * 1024],
                        psb[:].rearrange("b t n -> b (t n)"))

                # ---- transposes into PSUM: psT[sg] [128 s, (jj 8, b 48)]
                psTs = []
                for sg in range(2):
                    psT = ps_t.tile([128, 8, 48], bf16, tag="psT")
                    for jj in range(8):
                        jc = jj * 2 + sg
                        nc.tensor.transpose(
                            psT[:, jj, 0:B],
                            Mb[:, jc * 128:(jc + 1) * 128], e32_sb[:])
                        # ring-extend b by 16 (PSUM bf16 writes need matmul)
                        nc.tensor.transpose(
                            psT[:, jj, B:B + 16],
                            Mb[0:16, jc * 128:(jc + 1) * 128], e16_sb[:])
                    # DVE may read only one PSUM operand per instruction:
                    # stage the (broadcast) b1-side into SBUF via Act.
                    mts = dwp.tile([128, 8, B], bf16, tag=f"mts{sg}")
                    nc.scalar.copy(mts[:], psT[:, :, 0:B])
                    psTs.append((psT, mts))

                # ---- lagged sigma + deconv stages (keeps Act/PE queues hot)
                if g >= 1:
                    _sigma_stage(g - 1, dists[g - 1][:])
                    dists[g - 1] = None
                    _deconv_g(g - 1)

                # ---- DVE pairwise: 512 cyclic ordered pairs
                Dp = dwp.tile([128, 8, NO, B], bf16, tag="Dp")
                Dm = dwp.tile([128, 8, NO, B], bf16, tag="Dm")
                aDp = dwp.tile([128, 8, NO, B], bf16, tag="aDp")
                aDm = dwp.tile([128, 8, NO, B], bf16, tag="aDm")
                u = dwp.tile([128, 8, NO, B], bf16, tag="u")
                with nc.allow_low_precision("bf16 pairwise tree, f32 final"):
                    for sg, Dt in ((0, Dp), (1, Dm)):
                        a = psTs[sg][0][:]
                        in0 = (psTs[sg][1][:].unsqueeze(2)
                               .broadcast_to([128, 8, NO, B]))
                        in1 = AP(a.tensor, a.offset + 1,
                                 [list(a.ap[0]), [48, 8], [1, NO], [1, B]])
                        nc.vector.tensor_tensor(Dt[:], in0, in1, ALU.subtract)
                    nc.scalar.activation(aDp[:], Dp[:], AFT.Abs)
                    mbc = (msk_sb[:].unsqueeze(1).unsqueeze(1)
                           .broadcast_to([128, 8, NO, B]))
                    nc.vector.tensor_tensor(
                        aDm[:].bitcast(u16), Dm[:].bitcast(u16), mbc,
                        ALU.bitwise_and)
                    nc.vector.tensor_tensor(u[:], aDp[:], aDm[:], ALU.max)
                    uv = u[:].rearrange("p j o b -> p j (o b)")
                    v2t = dwp.tile([128, 2, 2, NPAIR], bf16, tag="v2")
                    nc.vector.tensor_tensor(
                        v2t[:].rearrange("p a c n -> p (a c) n"),
                        uv[:, 0:4, :], uv[:, 4:8, :], ALU.add)
                    w2 = dwp.tile([128, 2, NPAIR], bf16, tag="w2")
                    nc.vector.tensor_tensor(w2[:], v2t[:, 0], v2t[:, 1], ALU.add)
                    dist = ewp.tile([128, NPAIR], f32, tag="dist")
                    nc.vector.tensor_tensor(dist[:], w2[:, 0], w2[:, 1], ALU.add)
                dists[g] = dist

            _sigma_stage(NG - 1, dists[NG - 1][:])
            _deconv_g(NG - 1)

    nc.finalize()
    return nc


def _np_f8():
    import concourse.mybir as mybir
    return mybir.dt.np(mybir.dt.float8e4)


def _host_prep(x, w_conv, T, w_deconv):
    """Host-side prep: conv, T+- transform, scaling, fp8 casts, shards."""
    bf = ml_dtypes.bfloat16
    f8 = _np_f8()

    # conv (stride 4, VALID) on host: x_r (32, 768) f32
    xw = np.ascontiguousarray(x, dtype=np.float32).reshape(B, IN_FLT, 16, K, 16, K)
    x_r = np.einsum('bcirjs,ocrs->boij', xw, w_conv.astype(np.float32),
                    optimize=True).reshape(B, D_IN)
    xr3 = (2.0 * x_r.T).reshape(KCH, 128, B).transpose(1, 0, 2)
    xr8_host = np.ascontiguousarray(xr3).reshape(128, KCH * B).astype(f8)

    # deconv weights: lhsT[ic, (u*32+oc)*4+v] = w_deconv[oc, ic, u, v]
    wd_host = np.ascontiguousarray(
        np.transpose(w_deconv, (1, 2, 0, 3)).reshape(OC, 512)).astype(bf)

    # cyclic-pair incidence: pair p = (o-1)*32 + b1, o = 1..16
    inc_host = np.zeros((128, 4 * B), np.float32)
    for o in range(1, NO + 1):
        for b1 in range(B):
            p = (o - 1) * B + b1
            q, pw = p // 128, p % 128
            inc_host[pw, q * B + b1] = 1.0
            if o < NO:
                inc_host[pw, q * B + (b1 + o) % B] = 1.0
    inc_host = inc_host.astype(bf)
    eye32_host = np.eye(B, dtype=np.float32).astype(bf)
    eye128_host = np.eye(128, dtype=np.float32).astype(bf)
    eye16_host = np.eye(16, dtype=np.float32).astype(bf)
    msk_host = np.full((128, B), 0x7fff, np.uint16)

    # T: (768, 8192, 16) -> (768, 32oc, 16i, 16j, 16f)
    Tr = np.ascontiguousarray(T).reshape(D_IN, OC, T_SP, T_SP, F)
    in_maps = []
    for c in range(N_CORES):
        tslice = Tr[:, :, 2 * c:2 * c + 2, :, :]           # (768, oc, r, j, f)
        # -> (768, rj 32, oc 32, f 16) -> (768, g 8, x 4, oc 32, jj 8, 2)
        A = np.transpose(tslice, (0, 2, 3, 1, 4)).reshape(
            D_IN, NG, 4, OC, 8, 2)
        Apm = np.stack([A[..., 0] + A[..., 1], A[..., 0] - A[..., 1]], axis=4)
        # (768, g, x, oc, sg 2, jj 8) -> order cols (jj, sg, x, oc)
        arr = Apm.transpose(1, 0, 5, 4, 2, 3)   # (g, k, jj, sg, x, oc)
        ts6 = (64.0 * arr).reshape(NG, 3, 2, 128, 2048)  # (g, kp, i, p, col)
        tsh8_host = np.ascontiguousarray(
            ts6.transpose(0, 1, 3, 2, 4)).astype(f8).reshape(NG * 3 * 128, 4096)
        in_maps.append({
            "tsh8": tsh8_host,
            "xr8": xr8_host,
            "inc2": inc_host,
            "wd": wd_host,
            "eye32": eye32_host,
            "eye128": eye128_host,
            "eye16": eye16_host,
            "mskd": msk_host,
        })
    return in_maps


def _get_nc():
    if "nc" not in _CACHE:
        _CACHE["nc"] = _build_nc()
    return _CACHE["nc"]


def run(inputs, trace=False, trace_kwargs=None):
    """Run on hardware; returns (full_output, BassKernelResults)."""
    from concourse.bass_utils import run_bass_kernel_spmd
    nc = _get_nc()
    in_maps = _host_prep(inputs["x"], inputs["w_conv"], inputs["T"],
                         inputs["w_deconv"])
    res = run_bass_kernel_spmd(nc, in_maps, list(range(N_CORES)), trace=trace,
                               **(trace_kwargs or {}))
    x = np.asarray(inputs["x"], dtype=np.float32)
    full = np.empty((B, IN_FLT + OC, N, N), np.float32)
    full[:, :IN_FLT] = x
    for c in range(N_CORES):
        yc = np.asarray(res.results[c]["y"], dtype=np.float32)  # (256, 2048)
        for r in range(2):
            arr = yc[r * 128:(r + 1) * 128].reshape(4, 32, 4, 16, 32)
            # [u, o, v, j, b] -> [b, o, u, (j v)]
            full[:, IN_FLT:, 8 * c + 4 * r:8 * c + 4 * r + 4, :] = (
                arr.transpose(4, 1, 0, 3, 2).reshape(B, OC, 4, N))
    return full, res


def kernel(**inputs) -> np.ndarray:
    out, _ = run(inputs, trace=False)
    return out
